# revision 24
# baseline (speedup 1.0000x reference)
"""Differential attention kernel for Trainium2, 8-core SPMD (v2).

Problem (hardcoded shapes): B=2, S=2048, D=2048, H=16 heads, head_dim=128,
dual-chunk q/k dim 64.  out = (softmax(q1k1*s+m) - lam*softmax(q2k2*s+m)) @ v,
then output projection.

Sharding: batch x head-group.  Core c handles batch c//4 and heads
(c%4)*4 .. +4: its 4 heads' QKV columns (tensor parallel on c_attn output
cols), full attention for those heads, and a partial output projection
(tensor parallel on c_proj input rows).  The 4 partial projections per
batch are summed on host.

v2 design (vs v1 which computed scores transposed [k,q] and paid two PV
streams + two ones-matmul denominator streams per e-pair):

  - scores in [q,k] layout: per 128-q tile, stationary = q12 chunk,
    moving = k12, row-group packed across the two dual chunks.
  - exp on ScalarE with accum_out: softmax denominators come free as
    per-partition [128,1] sums (no ones-matmul denominator streams).
  - the two chains are merged BEFORE PV on VectorE:
        f = e1 * (1/d1) + e2 * (-lam/d2)
    (two tensor_scalar ops at 4x DVE rate + one in-place tensor_add at
    2x), so PV runs ONCE on f instead of twice on e1/e2.
  - f [q,k] is transposed to fT [k,q] by the DMA XBAR transpose
    (idle during attention), then a single PV matmul per k-chunk
    produces o^T = [hd, q] directly in the layout the projection wants.
  - head-major software pipeline: QKV for head h+1 (and the V c-tiles)
    are interleaved into attention of head h, so ScalarE's exp stream -
    the irreducible ~270us floor - overlaps the QKV matmuls instead of
    serializing after them.  X^T is re-streamed from DRAM once per head
    pass (4x input reads; SBUF can't hold X^T alongside the pipeline).

PE per core drops from ~500us (v1) to ~330us of 512-row bf16 matmul
streams; ScalarE ~300us (exp + accumulator reads) runs concurrently.

All matmul operands bf16 (error ~5e-3 vs the 2e-2 gate; fp8 variants
were measured over the gate in v1 tuning and rejected).
"""

import ml_dtypes
import numpy as np

import concourse.bass as bass
import concourse.mybir as mybir
import concourse.tile as tile
from concourse import bacc
from concourse.bass_utils import run_bass_kernel_spmd

F32 = mybir.dt.float32
BF16 = mybir.dt.bfloat16
EXP = mybir.ActivationFunctionType.Exp
A = mybir.AluOpType

B, S, D, H = 2, 2048, 2048, 16
HD = D // H            # 128 full head dim
QD = HD // 2           # 64 dual-chunk q/k dim
N_CORES = 8
HPC = H // (N_CORES // B)   # 4 heads per core
CPB = N_CORES // B          # 4 cores per batch
SCALE = float(HD) ** -0.5
EBIAS = -1.5           # exp(s*scale + EBIAS): cancels in softmax
KC = S // 128          # 16 k-chunks
DC = D // 128          # 16 d-chunks
NQT = S // 128         # 16 q-tiles per head
SCW = 512              # s-chunk width for QKV streaming


def build_program(lam: float, mask_trivial: bool):
    nc = bacc.Bacc("TRN2", target_bir_lowering=False, debug=False,
                   enable_asserts=False, num_devices=N_CORES)

    xt = nc.dram_tensor("xt", [D, S], BF16, kind="ExternalInput").ap()
    wqk = nc.dram_tensor("wqk", [D, 2 * HPC * HD], BF16, kind="ExternalInput").ap()
    wv = nc.dram_tensor("wv", [D, HPC * HD], BF16, kind="ExternalInput").ap()
    wp = nc.dram_tensor("wp", [HPC * HD, D], BF16, kind="ExternalInput").ap()
    maskb = nc.dram_tensor("maskb", [1, S], F32, kind="ExternalInput").ap()
    y = nc.dram_tensor("y", [S, D], F32, kind="ExternalOutput").ap()

    with tile.TileContext(nc) as tc:
        cms = {}

        def open_pool(**kw):
            cm = tc.tile_pool(**kw)
            cms[kw["name"]] = cm
            return cm.__enter__()

        def close_pool(name):
            cms.pop(name).__exit__(None, None, None)

        cpool = open_pool(name="consts", bufs=1)
        qkvres = open_pool(name="qkvres", bufs=1)
        qkpool = open_pool(name="qkp", bufs=2)
        wvp = open_pool(name="wvp", bufs=1)
        wpp = open_pool(name="wpp", bufs=1)
        xtp = open_pool(name="xtp", bufs=2)
        wqkp = open_pool(name="wqkp", bufs=2)
        epool = open_pool(name="ep", bufs=2)
        dpool = open_pool(name="dp", bufs=3)
        upool = open_pool(name="up", bufs=1)
        fpool = open_pool(name="fp", bufs=3)
        ftpool = open_pool(name="ftp", bufs=3)
        spsum = open_pool(name="sps", bufs=1, space="PSUM")   # s1 wide, 4 banks
        s2ps = open_pool(name="s2ps", bufs=1, space="PSUM")   # s2 half, 2 banks
        mps = open_pool(name="mps", bufs=2, space="PSUM")     # kq/v/pv, 2 banks

        ebias_t = cpool.tile([128, 1], F32, tag="ebias")
        nc.gpsimd.memset(ebias_t[:], EBIAS)
        if not mask_trivial:
            mrow_f = cpool.tile([1, S], F32, tag="mrowf")
            nc.sync.dma_start(mrow_f[:], maskb)
            mrow = cpool.tile([1, S], BF16, tag="mrow")
            nc.vector.tensor_copy(mrow[:], mrow_f[:])
            mones = cpool.tile([1, 128], BF16, tag="mones")
            nc.gpsimd.memset(mones[:], 1.0)

        # q12/k12 rotate through a 2-deep pool: head h's tiles are dead
        # once its scores finish, while head h+1's are being written
        q12, k12 = {}, {}

        def alloc_qk(h):
            q12[h] = qkpool.tile([128, S], BF16, tag="q12", name=f"q12_{h}")
            k12[h] = qkpool.tile([128, S], BF16, tag="k12", name=f"k12_{h}")

        g_t = []
        for h in range(HPC):
            g_t.append(qkvres.tile([HD, S], BF16, tag=f"g{h}", name=f"g{h}"))
        v_all = qkvres.tile([128, KC, HPC * HD], BF16, tag="v_all")
        wv_t = wvp.tile([128, DC, HPC * HD], BF16, tag="wv")
        wp_t = []

        # ---------------- QKV / V work units ----------------

        # bulk input loads go through the gpsimd SWDGE queue so the sync
        # queue carries only the latency-critical XBAR transposes
        def dma_xt_chunk(sc):
            t = xtp.tile([128, DC, SCW], BF16, tag="xt", name=f"xt_{sc}")
            for i in range(4):
                sl = slice(i * 4, (i + 1) * 4)
                nc.gpsimd.dma_start(
                    t[:, sl, :],
                    xt[i * 4 * 128:(i + 1) * 4 * 128,
                       sc * SCW:(sc + 1) * SCW]
                    .rearrange("(c p) s -> p c s", p=128))
            return t

        def load_wqk(ct):
            w = wqkp.tile([128, DC, 128], BF16, tag="wqk", name=f"wqk_{ct}")
            nc.gpsimd.dma_start(
                w[:], wqk[:, ct * 128:(ct + 1) * 128]
                .rearrange("(c p) m -> p c m", p=128))
            return w

        # QKV work is emitted in ~0.9us pieces so ready score pairs never
        # sit long behind a filler in the in-order PE queue
        def kq_unit_piece(h, which, xc, w_t, sc, piece, state):
            if piece == 0:
                state["ps"] = mps.tile([128, 512], F32, tag="m", name="mps_t")
            ps = state["ps"]
            for dc in range(piece * 4, piece * 4 + 4):
                nc.tensor.matmul(ps[:], w_t[:, dc, :], xc[:, dc, :],
                                 start=(dc == 0), stop=(dc == DC - 1))
            if piece == 3:
                dst = k12[h] if which == "k" else q12[h]
                nc.vector.tensor_copy(dst[:, sc * SCW:(sc + 1) * SCW],
                                      ps[:])

        def v_unit_piece(pair, xc, sc, st, piece, state):
            if piece == 0:
                state["ps"] = mps.tile([128, 512], F32, tag="m", name="mps_t")
            ps = state["ps"]
            for dc in range(piece * 8, piece * 8 + 8):
                nc.tensor.matmul(
                    ps[:, 0:256],
                    xc[:, dc, st * 128:(st + 1) * 128],
                    wv_t[:, dc, pair * 256:(pair + 1) * 256],
                    start=(dc == 0), stop=(dc == DC - 1))
            if piece == 1:
                nc.vector.tensor_copy(
                    v_all[:, sc * 4 + st, pair * 256:(pair + 1) * 256],
                    ps[:, 0:256])

        # filler queue: (cost_us, fn) pulled between score passes
        fillers = []

        def pull(budget):
            while fillers and budget > 0.0:
                c, fn = fillers.pop(0)
                fn()
                budget -= c

        def queue_pass(h_next, pairs):
            """One xt streaming pass: V units for `pairs` (first, PV needs
            them soonest) then k/q c-tile units for head h_next (if any)."""
            state = {}

            def ensure_sc(sc):
                key = f"xc_{sc}"
                if key not in state:
                    if h_next is not None and "wk" not in state:
                        alloc_qk(h_next)
                        state["wk"] = load_wqk(HPC + h_next)
                        state["wq"] = load_wqk(h_next)
                    state[key] = dma_xt_chunk(sc)
                return state[key]

            for sc in range(4):
                for pair in pairs:
                    for st in range(4):
                        vstate = {}
                        for piece in range(2):
                            def v_fn(pair=pair, sc=sc, st=st, piece=piece,
                                     vstate=vstate):
                                v_unit_piece(pair, ensure_sc(sc), sc, st,
                                             piece, vstate)
                            fillers.append((0.9, v_fn))
                if h_next is not None:
                    for which, wkey in (("k", "wk"), ("q", "wq")):
                        ustate = {}
                        for piece in range(4):
                            def kq_fn(which=which, wkey=wkey, sc=sc,
                                      piece=piece, ustate=ustate):
                                kq_unit_piece(h_next, which, ensure_sc(sc),
                                              state[wkey], sc, piece, ustate)
                            fillers.append((0.9, kq_fn))

        # ---------------- attention per qtile ----------------

        def emit_qtile(h, j, ft_tile):
            js = slice(j * 128, (j + 1) * 128)
            e12 = epool.tile([128, 2, S], BF16, tag="e12")
            bias = ebias_t[:] if mask_trivial else 0.0
            d1t = dpool.tile([128, 1], F32, tag="d1t")
            d2ab = []
            # chain-1 scores accumulate into one wide psum (single exp +
            # single accumulator read); chain-2 in two halves
            s1 = spsum.tile([128, 2048], F32, tag="s1")
            for p in range(2):
                s2 = s2ps.tile([128, 1024], F32, tag="s2")
                for kk in range(2):
                    ksl = slice(p * 1024 + kk * 512,
                                p * 1024 + (kk + 1) * 512)
                    osl = slice(kk * 512, (kk + 1) * 512)
                    nc.tensor.matmul(s1[:, ksl], q12[h][0:QD, js],
                                     k12[h][0:QD, ksl],
                                     start=True, stop=mask_trivial)
                    nc.tensor.matmul(s2[:, osl], q12[h][QD:128, js],
                                     k12[h][QD:128, ksl],
                                     start=True, stop=mask_trivial)
                    if not mask_trivial:
                        nc.tensor.matmul(s1[:, ksl], mones[:], mrow[:, ksl],
                                         start=False, stop=True)
                        nc.tensor.matmul(s2[:, osl], mones[:], mrow[:, ksl],
                                         start=False, stop=True)
                d2p = dpool.tile([128, 1], F32, tag=f"d2_{p}",
                                 name=f"d2_{p}")
                d2ab.append(d2p)
                nc.scalar.activation(e12[:, 1, p * 1024:(p + 1) * 1024],
                                     s2[:], EXP, bias=bias, scale=SCALE,
                                     accum_out=d2p[:])
                pull(1.7)
            nc.scalar.activation(e12[:, 0, :], s1[:], EXP,
                                 bias=bias, scale=SCALE, accum_out=d1t[:])
            # denominators -> per-partition scalars
            r1 = dpool.tile([128, 1], F32, tag="r1")
            nc.vector.reciprocal(r1[:], d1t[:])
            fa = upool.tile([128, S], BF16, tag="fa")
            nc.vector.tensor_scalar(fa[:], e12[:, 0, :], r1[:], None, A.mult)
            f = fpool.tile([128, S], BF16, tag="f")
            if lam != 0.0:
                d2 = dpool.tile([128, 1], F32, tag="d2")
                nc.vector.tensor_tensor(d2[:], d2ab[0][:], d2ab[1][:], A.add)
                dd2 = dpool.tile([128, 1], F32, tag="dd2")
                nc.vector.tensor_scalar(dd2[:], d2[:], -1.0 / lam, None,
                                        A.mult)
                s2 = dpool.tile([128, 1], F32, tag="s2")
                nc.vector.reciprocal(s2[:], dd2[:])
                u2 = upool.tile([128, S], BF16, tag="u2")
                nc.vector.tensor_scalar(u2[:], e12[:, 1, :], s2[:], None,
                                        A.mult)
                nc.vector.tensor_add(f[:], fa[:], u2[:])
            else:
                nc.vector.tensor_copy(f[:], fa[:])
            # transpose f [q,k] -> fT [k-part, kc, q-slice] via DMA XBAR
            nc.sync.dma_start(ft_tile[:, :, (j % 4) * 128:(j % 4 + 1) * 128],
                              f[:], transpose=True)

        def emit_pv(h, g, ft_tile):
            o = mps.tile([128, 512], F32, tag="m")
            for kc in range(KC):
                nc.tensor.matmul(o[:], v_all[:, kc, h * HD:(h + 1) * HD],
                                 ft_tile[:, kc, :],
                                 start=(kc == 0), stop=(kc == KC - 1))
            nc.vector.tensor_copy(g_t[h][:, g * 512:(g + 1) * 512], o[:])

        # ---------------- the pipeline ----------------

        # pre-phase: QKV k/q for head 0 (sequential, no attention yet)
        alloc_qk(0)
        wk0 = load_wqk(HPC + 0)
        wq0 = load_wqk(0)
        nc.gpsimd.dma_start(wv_t[:], wv.rearrange("(c p) n -> p c n", p=128))
        for sc in range(4):
            xc = dma_xt_chunk(sc)
            for which, wt in (("k", wk0), ("q", wq0)):
                st8 = {}
                for piece in range(4):
                    kq_unit_piece(0, which, xc, wt, sc, piece, st8)

        pv_pending = []   # (h, g, ft_tile) awaiting emission
        ft_cur = {}

        def drain_pv():
            # head 0's PV lags 2 groups (V pair 0 streams during step 0);
            # later heads lag 1 group
            while pv_pending:
                lag = 2 if pv_pending[0][0] == 0 else 1
                if len(pv_pending) <= lag:
                    break
                ph, pg, pft = pv_pending.pop(0)
                emit_pv(ph, pg, pft)

        for h in range(HPC):
            if h == 0:
                queue_pass(1, pairs=[0])     # V pair 0 + k/q head 1
            elif h == 1:
                queue_pass(2, pairs=[1])     # V pair 1 + k/q head 2
            elif h == 2:
                queue_pass(3, pairs=[])      # k/q head 3
            w = wpp.tile([HD, D], BF16, tag=f"wp{h}", name=f"wp{h}")
            nc.gpsimd.dma_start(w[:], wp[h * HD:(h + 1) * HD, :])
            wp_t.append(w)

            for j in range(NQT):
                if j % 4 == 0:
                    ft_cur[(h, j // 4)] = ftpool.tile(
                        [128, KC, 512], BF16, tag="ft",
                        name=f"ft_{h}_{j // 4}")
                emit_qtile(h, j, ft_cur[(h, j // 4)])
                pull(2.3)
                if j % 4 == 3:
                    pv_pending.append((h, j // 4, ft_cur.pop((h, j // 4))))
                drain_pv()
            # next step's scores need this step's k/q pass complete
            pull(1e9)
        while pv_pending:
            ph, pg, pft = pv_pending.pop(0)
            emit_pv(ph, pg, pft)

        # free attention PSUM banks and SBUF pools before the projection
        # opens its pools (LIFO: innermost pools first)
        for name in ["mps", "s2ps", "sps", "ftp", "fp", "up", "dp", "ep",
                     "wqkp", "xtp"]:
            close_pool(name)

        # ---------------- output projection ----------------
        with (
            tc.tile_pool(name="yev", bufs=2) as yev,
            tc.tile_pool(name="proj_ps", bufs=4, space="PSUM") as ppsum,
        ):
            for st in range(S // 128):
                yt = yev.tile([128, D], F32, tag="yt")
                for et in range(D // 512):
                    ps = ppsum.tile([128, 512], F32, tag="ps")
                    for hh in range(HPC):
                        nc.tensor.matmul(
                            ps[:],
                            g_t[hh][:, st * 128:(st + 1) * 128],
                            wp_t[hh][:, et * 512:(et + 1) * 512],
                            start=(hh == 0), stop=(hh == HPC - 1))
                    nc.vector.tensor_copy(yt[:, et * 512:(et + 1) * 512],
                                          ps[:])
                nc.sync.dma_start(y[st * 128:(st + 1) * 128, :], yt[:])

        for name in reversed(list(cms)):
            close_pool(name)

    nc.compile()
    return nc


_PROGRAM_CACHE: dict = {}


def _get_program(lam: float, mask_trivial: bool):
    key = (round(lam, 9), mask_trivial)
    if key not in _PROGRAM_CACHE:
        _PROGRAM_CACHE[key] = build_program(lam, mask_trivial)
    return _PROGRAM_CACHE[key]


def make_in_maps(hidden_states, attention_mask, W_attn, b_attn, W_proj):
    in_maps = []
    for c in range(N_CORES):
        b = c // CPB
        h0 = (c % CPB) * HPC
        xt = np.ascontiguousarray(hidden_states[b].T)
        cols = []
        for h in range(h0, h0 + HPC):
            cols.append(W_attn[:, h * QD:(h + 1) * QD])              # q1
            cols.append(W_attn[:, D // 2 + h * QD:D // 2 + (h + 1) * QD])  # q2
        for h in range(h0, h0 + HPC):
            cols.append(W_attn[:, D + h * QD:D + (h + 1) * QD])      # k1
            cols.append(W_attn[:, D + D // 2 + h * QD:D + D // 2 + (h + 1) * QD])
        wqk = np.ascontiguousarray(np.concatenate(cols, axis=1))
        wv = np.ascontiguousarray(W_attn[:, 2 * D + h0 * HD:2 * D + (h0 + HPC) * HD])
        wpm = np.ascontiguousarray(W_proj[h0 * HD:(h0 + HPC) * HD, :])
        maskb = np.ascontiguousarray(
            ((1.0 - attention_mask[b]) * -10000.0 + EBIAS).reshape(1, S)
        ).astype(np.float32)
        in_maps.append({
            "xt": xt.astype(ml_dtypes.bfloat16),
            "wqk": wqk.astype(ml_dtypes.bfloat16),
            "wv": wv.astype(ml_dtypes.bfloat16),
            "wp": wpm.astype(ml_dtypes.bfloat16),
            "maskb": maskb,
        })
    return in_maps


def kernel(hidden_states, attention_mask, W_attn, b_attn, W_proj, b_proj,
           lambda_param, _trace=False):
    hidden_states = np.asarray(hidden_states, np.float32)
    attention_mask = np.asarray(attention_mask, np.float32)
    W_attn = np.asarray(W_attn, np.float32)
    b_attn = np.asarray(b_attn, np.float32)
    W_proj = np.asarray(W_proj, np.float32)
    b_proj = np.asarray(b_proj, np.float32)
    lam = float(np.asarray(lambda_param))

    if np.any(b_attn != 0.0):
        raise NotImplementedError("nonzero b_attn not supported")

    mask_trivial = bool(np.all(attention_mask == 1.0))
    nc = _get_program(lam, mask_trivial)
    in_maps = make_in_maps(hidden_states, attention_mask, W_attn, b_attn,
                           W_proj)
    try:
        res = run_bass_kernel_spmd(nc, in_maps, core_ids=list(range(N_CORES)),
                                   trace=_trace)
    except ModuleNotFoundError:
        res = run_bass_kernel_spmd(nc, in_maps, core_ids=list(range(N_CORES)),
                                   trace=False)

    out = np.empty((B, S, D), np.float32)
    for b in range(B):
        acc = res.results[b * CPB]["y"].astype(np.float32).copy()
        for c in range(b * CPB + 1, (b + 1) * CPB):
            acc += res.results[c]["y"]
        out[b] = acc + b_proj[None, :]
    kernel.last_exec_time_ns = res.exec_time_ns
    if res.instructions_and_trace is not None:
        kernel.last_trace_path = res.instructions_and_trace[1]
    return out


kernel.last_exec_time_ns = None
kernel.last_trace_path = None
